# revision 7
# baseline (speedup 1.0000x reference)
"""MoE block (RMSNorm + top-4 router + 32-expert GLU FFN) on 8 TRN2 NeuronCores.

Expert-parallel: core c owns experts [4c, 4c+4). Each core redundantly
computes the RMSNorm + router over all 32 experts (fp16), then runs a dense
masked FFN over all 64 tokens for its own 4 experts with fp8 (e4m3) weights
(host-cast, scaled x32 with norm_w folded into w1/gate_w on the host).

The FFN avoids all PE transposes by computing h TRANSPOSED: MM1 uses the
128x128 fp8 w1 block as the stationary operand and the fp16 normed
activations (d on partitions) as the moving operand, so h lands in PSUM as
(i, t) tiles. b1 arrives via rank-1 bias matmuls. Activations then run on
full-128-partition tiles, the per-token routing weight is folded into h_act
via a PE-broadcast of A/(beta*32) (so all 4 experts' second matmuls plus the
b2 term accumulate into one shared PSUM group), and MM2 keeps w2 (fp8)
stationary producing out.T directly. A burst of dummy matmuls at t=0 warms
the PE HAM clock gate before the first expert's weights arrive.

The host sums the 8 partial (D, T) outputs and adds the residual.
"""

import sys
import types

sys.path.insert(0, "/opt/trn_rl_repo")

import numpy as np
import ml_dtypes

D = 640
I = 640
E = 32
T = 64
K = 4
EPS = 1e-5
LIMIT = 7.0
BETA = 1.702
NCORES = 8
EPC = E // NCORES          # experts per core
NCH = D // 128             # 5 contraction chunks of 128 (d = 5*p + c)
NBLK = (2 * I) // 128      # 10 i-blocks of 128 (i = 128*blk + p)
WS = 32.0                  # weight scale (fp8 range)
LIM_S = LIMIT * WS

TRACE = False
PROF_DIR = None
LAST_EXEC_NS = None

_NC = None


def _ensure_ntff_hook():
    """boot() skips NTFF hook registration (image antenv lacks axon_hooks);
    provide the module so bass_utils can profile when TRACE=True."""
    if "antenv.axon_hooks" in sys.modules:
        return
    try:
        from trn_agent_boot.trn_boot import _ntff_profile_via_ctypes
        hook = _ntff_profile_via_ctypes("/opt/axon/libaxon_pjrt.so")
    except Exception:
        hook = None
    m = types.ModuleType("antenv.axon_hooks")
    m.get_axon_ntff_profile_hook = lambda: hook
    m.set_axon_ntff_profile_hook = lambda h: None
    sys.modules["antenv.axon_hooks"] = m


def _build():
    import concourse.bass as bass
    import concourse.bacc as bacc
    import concourse.tile as tile
    from concourse import mybir

    f32 = mybir.dt.float32
    f16 = mybir.dt.float16
    f8 = mybir.dt.float8e4
    AF = mybir.ActivationFunctionType
    OP = mybir.AluOpType

    nc = bacc.Bacc("TRN2", target_bir_lowering=False, debug=False,
                   num_devices=NCORES)
    dx = nc.dram_tensor("x", (128, NCH * T), f16, kind="ExternalInput")
    dgw = nc.dram_tensor("gw", (128, NCH * E), f16, kind="ExternalInput")
    dgb = nc.dram_tensor("gb", (T, E), f32, kind="ExternalInput")
    didh = nc.dram_tensor("idh", (T, T), f16, kind="ExternalInput")
    desel = nc.dram_tensor("esel", (K, K * 128), f16, kind="ExternalInput")
    dw1 = nc.dram_tensor("w1", (EPC, 128, NCH * 2 * I), f8,
                         kind="ExternalInput")
    db1a = nc.dram_tensor("b1a", (8, EPC * 128), f16, kind="ExternalInput")
    db1b = nc.dram_tensor("b1b", (2, EPC * 128), f16, kind="ExternalInput")
    dbsel = nc.dram_tensor("bsel", (8, 512), f16, kind="ExternalInput")
    dw2 = nc.dram_tensor("w2", (EPC, 128, NCH * D), f8, kind="ExternalInput")
    db2 = nc.dram_tensor("b2", (EPC, D), f16, kind="ExternalInput")
    dout = nc.dram_tensor("out", (128, NCH * T), f16, kind="ExternalOutput")

    with tile.TileContext(nc) as tc:
        with (
            tc.tile_pool(name="consts", bufs=1) as consts,
            tc.tile_pool(name="small", bufs=2) as small,
            tc.tile_pool(name="hpool", bufs=2) as hpool,
        ):
            # ---- DMA: weight stream split across all three DMA queues
            # (sync HWDGE + gpsimd SWDGE + scalar HWDGE) — a single HWDGE
            # ring tops out ~178 GB/s, HBM allows ~358 per core. Small
            # tensors ride the scalar ring first (x leads). ----
            w1r = dw1.ap().rearrange("e p (c i) -> e p c i", c=NCH)
            w2r = dw2.ap().rearrange("e p (c d) -> e p c d", c=NCH)
            w1a_tiles = [consts.tile([128, 3, 2 * I], f8, tag=f"w1a_{e}",
                                     name=f"w1a{e}") for e in range(EPC)]
            w1b_tiles = [consts.tile([128, 2, 2 * I], f8, tag=f"w1b_{e}",
                                     name=f"w1b{e}") for e in range(EPC)]
            w2_tiles = [consts.tile([128, NCH, D], f8, tag=f"w2_{e}",
                                    name=f"w2t{e}") for e in range(EPC)]
            for e in range(EPC):
                nc.sync.dma_start(out=w1a_tiles[e], in_=w1r[e][:, 0:3, :])
            for e in range(EPC):
                nc.gpsimd.dma_start(out=w1b_tiles[e], in_=w1r[e][:, 3:5, :])
                nc.gpsimd.dma_start(out=w2_tiles[e], in_=w2r[e])

            x_t = consts.tile([128, NCH, T], f16)
            nc.scalar.dma_start(out=x_t,
                                in_=dx.ap().rearrange("p (c t) -> p c t",
                                                      c=NCH))
            gw_t = consts.tile([128, NCH, E], f16)
            nc.scalar.dma_start(out=gw_t,
                                in_=dgw.ap().rearrange("p (c e) -> p c e",
                                                       c=NCH))
            gb_t = consts.tile([T, E], f32)
            nc.scalar.dma_start(out=gb_t, in_=dgb.ap())
            idh = consts.tile([T, T], f16)
            nc.scalar.dma_start(out=idh, in_=didh.ap())
            esel = consts.tile([K, K * 128], f16)
            nc.scalar.dma_start(out=esel, in_=desel.ap())
            bsel = consts.tile([8, 512], f16)
            nc.scalar.dma_start(out=bsel, in_=dbsel.ap())
            b1a = consts.tile([8, EPC * 128], f16)
            nc.scalar.dma_start(out=b1a, in_=db1a.ap())
            b1b = consts.tile([2, EPC * 128], f16)
            nc.scalar.dma_start(out=b1b, in_=db1b.ap())
            b2_t = consts.tile([EPC, D], f16)
            nc.scalar.dma_start(out=b2_t, in_=db2.ap())

            ones128 = consts.tile([128, 128], f16)
            nc.vector.memset(ones128, 1.0)
            eps_t = consts.tile([128, 1], f32)
            nc.vector.memset(eps_t, EPS)

            rstd = consts.tile([128, T], f32)
            normed = consts.tile([128, NCH, T], f16)
            a4t = consts.tile([K, T], f16)
            adt = consts.tile([K, T], f16)

            with tc.tile_pool(name="ps_misc", bufs=1, space="PSUM") as ps_m:
                # ---- HAM warm-up: keep the PE busy from t~0 so the clock
                # gate opens before the first expert weights land ----
                warm = ps_m.tile([128, 512], f32, tag="warm")
                for _ in range(14):
                    nc.tensor.matmul(warm[:, 0:128], ones128, ones128,
                                     start=True, stop=True)

                # ---- RMSNorm: d = 5p + c on partitions ----
                xx = small.tile([128, NCH, T], f16, tag="xx")
                nc.vector.tensor_mul(xx, x_t, x_t)
                ps_ss = ps_m.tile([128, T], f32, tag="ss")
                for c in range(NCH):
                    nc.tensor.matmul(ps_ss, ones128, xx[:, c, :],
                                     start=(c == 0), stop=(c == NCH - 1))
                sq = small.tile([128, T], f32, tag="sq")
                nc.scalar.activation(sq, ps_ss, AF.Sqrt, bias=eps_t,
                                     scale=1.0 / D)
                nc.vector.reciprocal(rstd, sq)
                rstd_b = bass.AP(tensor=rstd.tensor, offset=rstd.offset,
                                 ap=[rstd.ap[0], [0, NCH], [1, T]])
                nc.vector.tensor_mul(normed, x_t, rstd_b)

                # ---- router: gate, top-4, softmax, routing matrix A ----
                ps_g = ps_m.tile([T, E], f32, tag="g")
                for c in range(NCH):
                    nc.tensor.matmul(ps_g, normed[:, c, :], gw_t[:, c, :],
                                     start=(c == 0), stop=(c == NCH - 1))
                g_sb = small.tile([T, E], f32, tag="gsb")
                nc.vector.tensor_add(g_sb, ps_g, gb_t)

                m8 = small.tile([T, 8], f32, tag="m8")
                nc.vector.max(m8, g_sb)
                negm = small.tile([T, 1], f32, tag="negm")
                nc.vector.tensor_scalar_mul(negm, m8[:, 0:1], -1.0)
                s4 = small.tile([T, K], f32, tag="s4")
                nc.scalar.activation(s4, m8[:, 0:K], AF.Exp, bias=negm,
                                     scale=1.0)
                den = small.tile([T, 1], f32, tag="den")
                nc.vector.reduce_sum(den, s4, axis=mybir.AxisListType.X)
                rden = small.tile([T, 1], f32, tag="rden")
                nc.vector.reciprocal(rden, den)
                ew = small.tile([T, K], f32, tag="ew")
                nc.vector.tensor_scalar_mul(ew, s4, rden)

                A = small.tile([T, E], f32, tag="A")
                for k in range(K):
                    msk = small.tile([T, E], f32, tag="msk")
                    nc.vector.tensor_scalar(msk, g_sb, m8[:, k:k + 1], None,
                                            op0=OP.is_equal)
                    wm = small.tile([T, E], f32, tag="wm")
                    nc.vector.tensor_scalar_mul(wm, msk, ew[:, k:k + 1])
                    if k == 0:
                        nc.vector.tensor_copy(A, wm)
                    else:
                        nc.vector.tensor_add(A, A, wm)
                A_hf = small.tile([T, K], f16, tag="A_hf")
                nc.vector.tensor_copy(A_hf, A[:, 0:K])
                # h_act lands as 32*true value (gm keeps the x32 weight
                # scale) and w2 is host-scaled x32; fold 1/1024 into the
                # routing scale.
                Ad_hf = small.tile([T, K], f16, tag="Ad_hf")
                nc.vector.tensor_scalar_mul(Ad_hf, A[:, 0:K],
                                            1.0 / (WS * WS))
                ps_a4 = ps_m.tile([K, T], f16, tag="tr", bufs=2)
                nc.tensor.transpose(ps_a4, A_hf, idh)
                nc.vector.tensor_copy(a4t, ps_a4)
                ps_ad = ps_m.tile([K, T], f16, tag="tr", bufs=2)
                nc.tensor.transpose(ps_ad, Ad_hf, idh)
                nc.vector.tensor_copy(adt, ps_ad)

            # ---- experts: dense masked GLU FFN, all in (i, t) layout ----
            with (
                tc.tile_pool(name="ps_h", bufs=2, space="PSUM") as ps_h,
                tc.tile_pool(name="ps_o", bufs=1, space="PSUM") as ps_op,
            ):
                ps_o = ps_op.tile([128, NCH, T], f32, tag="o")
                ab_ps = ps_op.tile([128, K * T], f32, tag="ab")

                def emit_mm1(e):
                    hA = ps_h.tile([128, 512], f32, tag="hA")
                    hB = ps_h.tile([128, 128], f32, tag="hB")
                    # bias: one selector matmul per psum region
                    nc.tensor.matmul(hA, b1a[:, 128 * e:128 * e + 128],
                                     bsel, start=True, stop=False)
                    nc.tensor.matmul(hB, b1b[:, 128 * e:128 * e + 128],
                                     bsel[0:2, 0:128], start=True,
                                     stop=False)
                    for c in range(NCH):
                        for blk in range(NBLK):
                            pt, col = ((hA, 64 * blk) if blk < 8
                                       else (hB, 64 * (blk - 8)))
                            wsl = (w1a_tiles[e][:, c,
                                                128 * blk:128 * blk + 128]
                                   if c < 3 else
                                   w1b_tiles[e][:, c - 3,
                                                128 * blk:128 * blk + 128])
                            nc.tensor.matmul(
                                pt[:, col:col + 64], wsl,
                                normed[:, c, :], start=False,
                                stop=(c == NCH - 1 and blk in (7, 9)))
                    return hA, hB

                def emit_act(e, hA, hB):
                    # hA/hB hold 32*(t@w1'+b1) transposed: glu blocks 0-4,
                    # lin blocks 5-9. clip at 32*LIMIT, descale inside
                    # silu's scale and lin's x/32+1.
                    gm = hpool.tile([128, NCH * T], f16, tag="gm")
                    nc.vector.tensor_scalar(gm, hA[:, 0:320], LIM_S, None,
                                            op0=OP.min)
                    l1 = hpool.tile([128, NCH * T], f16, tag="l1")
                    nc.vector.tensor_scalar(l1[:, 0:192], hA[:, 320:512],
                                            LIM_S, -LIM_S,
                                            op0=OP.min, op1=OP.max)
                    nc.vector.tensor_scalar(l1[:, 192:320], hB[:, 0:128],
                                            LIM_S, -LIM_S,
                                            op0=OP.min, op1=OP.max)
                    l2 = hpool.tile([128, NCH * T], f16, tag="l2")
                    nc.vector.tensor_scalar(l2, l1, 1.0 / WS, 1.0,
                                            op0=OP.mult, op1=OP.add)
                    p_ = hpool.tile([128, NCH * T], f16, tag="p_")
                    nc.scalar.activation(p_, gm, AF.Sigmoid, scale=BETA / WS)
                    q = hpool.tile([128, NCH * T], f16, tag="q")
                    nc.vector.tensor_mul(q, gm, p_)
                    ha = hpool.tile([128, NCH, T], f16, tag="ha")
                    nc.vector.tensor_mul(
                        ha, q.rearrange("p (c t) -> p c t", c=NCH),
                        l2.rearrange("p (c t) -> p c t", c=NCH))
                    ha_s = hpool.tile([128, NCH, T], f16, tag="ha_s")
                    ab_b = bass.AP(tensor=ab_ps.tensor,
                                   offset=ab_ps.offset + T * e,
                                   ap=[ab_ps.ap[0], [0, NCH], [1, T]])
                    nc.vector.tensor_mul(ha_s, ha, ab_b)
                    return ha_s

                def emit_mm2(e, ha_s, last):
                    w2_t = w2_tiles[e]
                    for c in range(NCH):
                        for db in range(NCH):
                            nc.tensor.matmul(
                                ps_o[:, db, :],
                                w2_t[:, c, 128 * db:128 * db + 128],
                                ha_s[:, c, :], start=False,
                                stop=(last and c == NCH - 1
                                      and db == NCH - 1))

                h0 = emit_mm1(0)
                # A broadcast (one row per expert, all 128 partitions) and
                # the b2 base term — emitted after expert 0's MM1 so they
                # never delay it in the in-order PE stream.
                for e in range(EPC):
                    nc.tensor.matmul(ab_ps[:, T * e:T * e + T],
                                     esel[:, 128 * e:128 * e + 128], adt,
                                     start=(e == 0), stop=(e == EPC - 1))
                for db in range(NCH):
                    nc.tensor.matmul(ps_o[:, db, :],
                                     b2_t[:, 128 * db:128 * db + 128],
                                     a4t, start=(db == 0), stop=False)
                has0 = emit_act(0, *h0)
                h1 = emit_mm1(1)
                emit_mm2(0, has0, False)
                has1 = emit_act(1, *h1)
                h2 = emit_mm1(2)
                emit_mm2(1, has1, False)
                has2 = emit_act(2, *h2)
                h3 = emit_mm1(3)
                emit_mm2(2, has2, False)
                has3 = emit_act(3, *h3)
                emit_mm2(3, has3, True)

                out_sb = consts.tile([128, NCH * T], f16)
                nc.vector.tensor_copy(
                    out_sb, ps_o.rearrange("p c t -> p (c t)"))
            nc.sync.dma_start(out=dout.ap(), in_=out_sb)

    nc.finalize()
    return nc


def _get_nc():
    global _NC
    if _NC is None:
        _ensure_ntff_hook()
        _NC = _build()
    return _NC


def _prep_core(c, x2, norm_w, gate_w, gate_b, w1, b1, w2, b2):
    f16 = np.float16
    f8 = ml_dtypes.float8_e4m3
    lo, hi = EPC * c, EPC * (c + 1)
    perm = np.r_[lo:hi, 0:lo, hi:E]
    nw = norm_w.astype(np.float64)
    w1h = (w1[lo:hi].astype(np.float64) * nw[None, :, None] * WS)
    w1h = w1h.astype(np.float32).astype(f8)
    w1h = np.ascontiguousarray(
        w1h.reshape(EPC, 128, NCH, 2 * I)).reshape(EPC, 128, NCH * 2 * I)
    w2h = (w2[lo:hi] * np.float32(WS)).astype(f8)
    w2h = np.ascontiguousarray(
        w2h.reshape(EPC, NCH, 128, D).transpose(0, 2, 1, 3)
    ).reshape(EPC, 128, NCH * D)
    gwp = (gate_w[perm].astype(np.float64) * nw[None, :]).T
    gwp = np.ascontiguousarray(gwp.astype(f16)).reshape(128, NCH * E)
    b1s = (b1[lo:hi].astype(np.float32) * WS).astype(f16)
    b1a = np.ascontiguousarray(
        b1s[:, :1024].reshape(EPC, 8, 128).transpose(1, 0, 2)).reshape(
        8, EPC * 128)
    b1b = np.ascontiguousarray(
        b1s[:, 1024:].reshape(EPC, 2, 128).transpose(1, 0, 2)).reshape(
        2, EPC * 128)
    return {
        "x": np.ascontiguousarray(x2.astype(f16)).reshape(128, NCH * T),
        "gw": gwp,
        "gb": np.ascontiguousarray(
            np.tile(gate_b[perm][None, :], (T, 1)).astype(np.float32)),
        "idh": np.eye(T, dtype=f16),
        "esel": np.kron(np.eye(K), np.ones(128)).astype(f16),
        "bsel": np.kron(np.eye(8), np.ones(64)).astype(f16),
        "w1": w1h,
        "b1a": b1a,
        "b1b": b1b,
        "w2": w2h,
        "b2": b2[lo:hi].astype(f16),
    }


def kernel(**inputs):
    global LAST_EXEC_NS
    nc = _get_nc()
    from concourse.bass_utils import run_bass_kernel_spmd

    x = np.ascontiguousarray(np.asarray(inputs["x"], dtype=np.float32))
    norm_w = np.asarray(inputs["norm_w"], np.float32)
    gate_w = np.asarray(inputs["gate_w"], np.float32)
    gate_b = np.asarray(inputs["gate_b"], np.float32)
    w1 = np.asarray(inputs["w1"], np.float32)
    b1 = np.asarray(inputs["b1"], np.float32)
    w2 = np.asarray(inputs["w2"], np.float32)
    b2 = np.asarray(inputs["b2"], np.float32)

    x2 = np.ascontiguousarray(x[0, :, 0, :])  # (D, T)
    in_maps = [_prep_core(c, x2, norm_w, gate_w, gate_b, w1, b1, w2, b2)
               for c in range(NCORES)]

    res = run_bass_kernel_spmd(nc, in_maps, core_ids=list(range(NCORES)),
                               trace=TRACE, tmpdir=PROF_DIR)
    LAST_EXEC_NS = res.exec_time_ns
    total = np.zeros((D, T), np.float32)
    for r in res.results:
        o = np.asarray(r["out"], np.float32).reshape(128, NCH, T)
        total += o.transpose(1, 0, 2).reshape(D, T)
    return (x + total[None, :, None, :]).astype(np.float32)


# revision 9
# speedup vs baseline: 1.2358x; 1.2358x over previous
"""MoE block (RMSNorm + top-4 router + 32-expert GLU FFN) on 8 TRN2 NeuronCores.

Expert-parallel: core c owns experts [4c, 4c+4). Each core redundantly
computes the RMSNorm + router over all 32 experts (fp16), then runs a dense
masked FFN over all 64 tokens for its own 4 experts with fp8 (e4m3) weights
(host-cast, scaled x32 with norm_w folded into w1/gate_w on the host).

The FFN avoids all PE transposes by computing h TRANSPOSED: MM1 uses the
128x128 fp8 w1 block as the stationary operand and the fp16 normed
activations (d on partitions) as the moving operand, so h lands in PSUM as
(i, t) tiles. b1 arrives via two selector matmuls per expert at the end of
each accumulation group. Activations run on full-128-partition tiles, the
per-token routing weight is folded into h_act via a PE-broadcast of A/1024
(so all 4 experts' second matmuls plus the b2 term accumulate into one
shared PSUM group), and MM2 keeps w2 (fp8) stationary producing out.T.

DMA: one HWDGE/SWDGE queue sustains only ~180-230 GB/s (16 SDMA engines x
~20 GB/s x duty cycle, with ~1us inter-DMA receipt bubbles), so the weight
stream is split across all three queues: sync carries the leading w1
chunks, gpsimd the trailing w1 chunks, and the scalar engine issues the w2
loads interleaved between its activation ops. Small tensors are packed
into two block DMAs. A burst of dummy matmuls warms the PE HAM clock gate
before the first expert weights arrive.

The host sums the 8 partial (D, T) outputs and adds the residual.
"""

import sys
import types

sys.path.insert(0, "/opt/trn_rl_repo")

import numpy as np
import ml_dtypes

D = 640
I = 640
E = 32
T = 64
K = 4
EPS = 1e-5
LIMIT = 7.0
BETA = 1.702
NCORES = 8
EPC = E // NCORES          # experts per core
NCH = D // 128             # 5 contraction chunks of 128 (d = 5*p + c)
NBLK = (2 * I) // 128      # 10 i-blocks of 128 (i = 128*blk + p)
WS = 32.0                  # weight scale (fp8 range)
LIM_S = LIMIT * WS
SPLITS = [2, 3, 3, 3]      # w1 chunks on the sync queue, per expert

TRACE = False
PROF_DIR = None
LAST_EXEC_NS = None

_NC = None


def _ensure_ntff_hook():
    """boot() skips NTFF hook registration (image antenv lacks axon_hooks);
    provide the module so bass_utils can profile when TRACE=True."""
    if "antenv.axon_hooks" in sys.modules:
        return
    try:
        from trn_agent_boot.trn_boot import _ntff_profile_via_ctypes
        hook = _ntff_profile_via_ctypes("/opt/axon/libaxon_pjrt.so")
    except Exception:
        hook = None
    m = types.ModuleType("antenv.axon_hooks")
    m.get_axon_ntff_profile_hook = lambda: hook
    m.set_axon_ntff_profile_hook = lambda h: None
    sys.modules["antenv.axon_hooks"] = m


def _build():
    import concourse.bass as bass
    import concourse.bacc as bacc
    import concourse.tile as tile
    from concourse import mybir

    f32 = mybir.dt.float32
    f16 = mybir.dt.float16
    f8 = mybir.dt.float8e4
    AF = mybir.ActivationFunctionType
    OP = mybir.AluOpType

    nc = bacc.Bacc("TRN2", target_bir_lowering=False, debug=False,
                   num_devices=NCORES)
    # m128: x (0:320) | gw (320:480) | idh rows 0-63 (480:544) | gb (544:576)
    dm128 = nc.dram_tensor("m128", (128, 576), f16, kind="ExternalInput")
    # m8: esel r0-3 (0:512) | bsel (512:1024) | b1a (1024:1536) |
    #     b1b r0-1 (1536:2048) | b2 r0-3 (2048:2688)
    dm8 = nc.dram_tensor("m8t", (8, 2688), f16, kind="ExternalInput")
    dw1 = nc.dram_tensor("w1", (EPC, 128, NCH * 2 * I), f8,
                         kind="ExternalInput")
    dw2 = nc.dram_tensor("w2", (EPC, 128, NCH * D), f8, kind="ExternalInput")
    dout = nc.dram_tensor("out", (128, NCH * T), f16, kind="ExternalOutput")

    with tile.TileContext(nc) as tc:
        with (
            tc.tile_pool(name="consts", bufs=1) as consts,
            tc.tile_pool(name="small", bufs=2) as small,
            tc.tile_pool(name="hpool", bufs=2) as hpool,
        ):
            # ---- DMA: sync leads with the packed small block + early w1
            # chunks; gpsimd carries trailing w1 chunks; w2 issues ride the
            # scalar engine between activation ops (see below). ----
            w1r = dw1.ap().rearrange("e p (c i) -> e p c i", c=NCH)
            w2r = dw2.ap().rearrange("e p (c d) -> e p c d", c=NCH)
            m128 = consts.tile([128, 576], f16)
            nc.sync.dma_start(out=m128, in_=dm128.ap())
            w1a_tiles = [consts.tile([128, SPLITS[e], 2 * I], f8,
                                     tag=f"w1a_{e}", name=f"w1a{e}")
                         for e in range(EPC)]
            w1b_tiles = [consts.tile([128, NCH - SPLITS[e], 2 * I], f8,
                                     tag=f"w1b_{e}", name=f"w1b{e}")
                         for e in range(EPC)]
            w2_tiles = [consts.tile([128, NCH, D], f8, tag=f"w2_{e}",
                                    name=f"w2t{e}") for e in range(EPC)]
            for e in range(EPC):
                nc.sync.dma_start(out=w1a_tiles[e],
                                  in_=w1r[e][:, 0:SPLITS[e], :])
            m8t = consts.tile([8, 2688], f16)
            nc.gpsimd.dma_start(out=w1b_tiles[0],
                                in_=w1r[0][:, SPLITS[0]:NCH, :])
            nc.gpsimd.dma_start(out=m8t, in_=dm8.ap())
            for e in range(1, EPC):
                nc.gpsimd.dma_start(out=w1b_tiles[e],
                                    in_=w1r[e][:, SPLITS[e]:NCH, :])

            x3 = m128[:, 0:320].rearrange("p (c t) -> p c t", c=NCH)
            gw3 = m128[:, 320:480].rearrange("p (c e) -> p c e", c=NCH)
            idh = m128[0:T, 480:544]
            gbf = m128[0:T, 544:576]
            esel = m8t[0:K, 0:512]
            bsel = m8t[:, 512:1024]
            b1a = m8t[:, 1024:1536]
            b1b = m8t[0:2, 1536:2048]
            b2_t = m8t[0:K, 2048:2688]

            ones128 = consts.tile([128, 128], f16)
            nc.vector.memset(ones128, 1.0)
            eps_t = consts.tile([128, 1], f32)
            nc.vector.memset(eps_t, EPS)
            # preload the Sqrt table (ACT holds one table; Sqrt is used
            # first — Exp and Sigmoid load inline at first use)
            dmy = consts.tile([1, 1], f32)
            nc.scalar.activation(dmy, eps_t[0:1, :], AF.Sqrt)

            rstd = consts.tile([128, T], f32)
            normed = consts.tile([128, NCH, T], f16)
            a4t = consts.tile([K, T], f16)
            adt = consts.tile([K, T], f16)

            with tc.tile_pool(name="ps_misc", bufs=1, space="PSUM") as ps_m:
                # ---- HAM warm-up: keep the PE busy early so the clock
                # gate opens before the first expert weights land ----
                warm = ps_m.tile([128, 512], f32, tag="warm")
                for _ in range(8):
                    nc.tensor.matmul(warm[:, 0:128], ones128, ones128,
                                     start=True, stop=True)

                # ---- RMSNorm: d = 5p + c on partitions ----
                xx = small.tile([128, NCH, T], f16, tag="xx")
                nc.vector.tensor_mul(xx, x3, x3)
                ps_ss = ps_m.tile([128, T], f32, tag="ss")
                for c in range(NCH):
                    nc.tensor.matmul(ps_ss, ones128, xx[:, c, :],
                                     start=(c == 0), stop=(c == NCH - 1))
                sq = small.tile([128, T], f32, tag="sq")
                nc.scalar.activation(sq, ps_ss, AF.Sqrt, bias=eps_t,
                                     scale=1.0 / D)
                nc.scalar.dma_start(out=w2_tiles[0], in_=w2r[0])
                nc.vector.reciprocal(rstd, sq)
                rstd_b = bass.AP(tensor=rstd.tensor, offset=rstd.offset,
                                 ap=[rstd.ap[0], [0, NCH], [1, T]])
                nc.vector.tensor_mul(normed, x3, rstd_b)

                # ---- router: gate, top-4, softmax, routing matrix A ----
                ps_g = ps_m.tile([T, E], f32, tag="g")
                for c in range(NCH):
                    nc.tensor.matmul(ps_g, normed[:, c, :], gw3[:, c, :],
                                     start=(c == 0), stop=(c == NCH - 1))
                g_sb = small.tile([T, E], f32, tag="gsb")
                nc.vector.tensor_add(g_sb, ps_g, gbf)

                m8 = small.tile([T, 8], f32, tag="m8")
                nc.vector.max(m8, g_sb)
                negm = small.tile([T, 1], f32, tag="negm")
                nc.vector.tensor_scalar_mul(negm, m8[:, 0:1], -1.0)
                s4 = small.tile([T, K], f32, tag="s4")
                nc.scalar.activation(s4, m8[:, 0:K], AF.Exp, bias=negm,
                                     scale=1.0)
                nc.scalar.dma_start(out=w2_tiles[1], in_=w2r[1])
                den = small.tile([T, 1], f32, tag="den")
                nc.vector.reduce_sum(den, s4, axis=mybir.AxisListType.X)
                rden = small.tile([T, 1], f32, tag="rden")
                nc.vector.reciprocal(rden, den)
                ew = small.tile([T, K], f32, tag="ew")
                nc.vector.tensor_scalar_mul(ew, s4, rden)

                A = small.tile([T, E], f32, tag="A")
                for k in range(K):
                    msk = small.tile([T, E], f32, tag="msk")
                    nc.vector.tensor_scalar(msk, g_sb, m8[:, k:k + 1], None,
                                            op0=OP.is_equal)
                    wm = small.tile([T, E], f32, tag="wm")
                    nc.vector.tensor_scalar_mul(wm, msk, ew[:, k:k + 1])
                    if k == 0:
                        nc.vector.tensor_copy(A, wm)
                    else:
                        nc.vector.tensor_add(A, A, wm)
                A_hf = small.tile([T, K], f16, tag="A_hf")
                nc.vector.tensor_copy(A_hf, A[:, 0:K])
                # h_act lands as 32*true value (gm keeps the x32 weight
                # scale) and w2 is host-scaled x32; fold 1/1024 into the
                # routing scale.
                Ad_hf = small.tile([T, K], f16, tag="Ad_hf")
                nc.vector.tensor_scalar_mul(Ad_hf, A[:, 0:K],
                                            1.0 / (WS * WS))
                ps_a4 = ps_m.tile([K, T], f16, tag="tr", bufs=2)
                nc.tensor.transpose(ps_a4, A_hf, idh)
                nc.vector.tensor_copy(a4t, ps_a4)
                ps_ad = ps_m.tile([K, T], f16, tag="tr", bufs=2)
                nc.tensor.transpose(ps_ad, Ad_hf, idh)
                nc.vector.tensor_copy(adt, ps_ad)

            # ---- experts: dense masked GLU FFN, all in (i, t) layout ----
            with (
                tc.tile_pool(name="ps_h", bufs=2, space="PSUM") as ps_h,
                tc.tile_pool(name="ps_o", bufs=1, space="PSUM") as ps_op,
            ):
                ps_o = ps_op.tile([128, NCH, T], f32, tag="o")
                ab_ps = ps_op.tile([128, K * T], f32, tag="ab")

                def emit_mm1(e):
                    hA = ps_h.tile([128, 512], f32, tag="hA")
                    hB = ps_h.tile([128, 128], f32, tag="hB")
                    na = SPLITS[e]
                    for c in range(NCH):
                        for blk in range(NBLK):
                            pt, col = ((hA, 64 * blk) if blk < 8
                                       else (hB, 64 * (blk - 8)))
                            wsl = (w1a_tiles[e][:, c,
                                                128 * blk:128 * blk + 128]
                                   if c < na else
                                   w1b_tiles[e][:, c - na,
                                                128 * blk:128 * blk + 128])
                            nc.tensor.matmul(
                                pt[:, col:col + 64], wsl,
                                normed[:, c, :],
                                start=(c == 0 and blk in (0, 8)),
                                stop=False)
                    # bias last: one selector matmul per psum region, so
                    # the (late-arriving) b1 block never gates MM1's start
                    nc.tensor.matmul(hA, b1a[:, 128 * e:128 * e + 128],
                                     bsel, start=False, stop=True)
                    nc.tensor.matmul(hB, b1b[:, 128 * e:128 * e + 128],
                                     bsel[0:2, 0:128], start=False,
                                     stop=True)
                    return hA, hB

                def emit_act(e, hA, hB):
                    # hA/hB hold 32*(t@w1'+b1) transposed: glu blocks 0-4,
                    # lin blocks 5-9. clip at 32*LIMIT, descale inside
                    # sigmoid's scale and lin's x/32+1.
                    gm = hpool.tile([128, NCH * T], f16, tag="gm")
                    nc.vector.tensor_scalar(gm, hA[:, 0:320], LIM_S, None,
                                            op0=OP.min)
                    l1 = hpool.tile([128, NCH * T], f16, tag="l1")
                    nc.vector.tensor_scalar(l1[:, 0:192], hA[:, 320:512],
                                            LIM_S, -LIM_S,
                                            op0=OP.min, op1=OP.max)
                    nc.vector.tensor_scalar(l1[:, 192:320], hB[:, 0:128],
                                            LIM_S, -LIM_S,
                                            op0=OP.min, op1=OP.max)
                    l2 = hpool.tile([128, NCH * T], f16, tag="l2")
                    nc.vector.tensor_scalar(l2, l1, 1.0 / WS, 1.0,
                                            op0=OP.mult, op1=OP.add)
                    p_ = hpool.tile([128, NCH * T], f16, tag="p_")
                    nc.scalar.activation(p_, gm, AF.Sigmoid, scale=BETA / WS)
                    q = hpool.tile([128, NCH * T], f16, tag="q")
                    nc.vector.tensor_mul(q, gm, p_)
                    ha = hpool.tile([128, NCH, T], f16, tag="ha")
                    nc.vector.tensor_mul(
                        ha, q.rearrange("p (c t) -> p c t", c=NCH),
                        l2.rearrange("p (c t) -> p c t", c=NCH))
                    ha_s = hpool.tile([128, NCH, T], f16, tag="ha_s")
                    ab_b = bass.AP(tensor=ab_ps.tensor,
                                   offset=ab_ps.offset + T * e,
                                   ap=[ab_ps.ap[0], [0, NCH], [1, T]])
                    nc.vector.tensor_mul(ha_s, ha, ab_b)
                    return ha_s

                def emit_mm2(e, ha_s, last):
                    w2_t = w2_tiles[e]
                    for c in range(NCH):
                        for db in range(NCH):
                            nc.tensor.matmul(
                                ps_o[:, db, :],
                                w2_t[:, c, 128 * db:128 * db + 128],
                                ha_s[:, c, :], start=False,
                                stop=(last and c == NCH - 1
                                      and db == NCH - 1))

                h0 = emit_mm1(0)
                # A broadcast (one row per expert, all 128 partitions) and
                # the b2 base term — emitted after expert 0's MM1 so they
                # never delay it in the in-order PE stream.
                for e in range(EPC):
                    nc.tensor.matmul(ab_ps[:, T * e:T * e + T],
                                     esel[:, 128 * e:128 * e + 128], adt,
                                     start=(e == 0), stop=(e == EPC - 1))
                for db in range(NCH):
                    nc.tensor.matmul(ps_o[:, db, :],
                                     b2_t[:, 128 * db:128 * db + 128],
                                     a4t, start=(db == 0), stop=False)
                has0 = emit_act(0, *h0)
                nc.scalar.dma_start(out=w2_tiles[2], in_=w2r[2])
                h1 = emit_mm1(1)
                emit_mm2(0, has0, False)
                has1 = emit_act(1, *h1)
                nc.scalar.dma_start(out=w2_tiles[3], in_=w2r[3])
                h2 = emit_mm1(2)
                emit_mm2(1, has1, False)
                has2 = emit_act(2, *h2)
                h3 = emit_mm1(3)
                emit_mm2(2, has2, False)
                has3 = emit_act(3, *h3)
                emit_mm2(3, has3, True)

                out_sb = consts.tile([128, NCH * T], f16)
                nc.vector.tensor_copy(
                    out_sb, ps_o.rearrange("p c t -> p (c t)"))
            nc.sync.dma_start(out=dout.ap(), in_=out_sb)

    nc.finalize()
    return nc


def _get_nc():
    global _NC
    if _NC is None:
        _ensure_ntff_hook()
        _NC = _build()
    return _NC


def _prep_core(c, x2, norm_w, gate_w, gate_b, w1, b1, w2, b2):
    f16 = np.float16
    f8 = ml_dtypes.float8_e4m3
    lo, hi = EPC * c, EPC * (c + 1)
    perm = np.r_[lo:hi, 0:lo, hi:E]
    nw = norm_w.astype(np.float64)
    w1h = (w1[lo:hi].astype(np.float64) * nw[None, :, None] * WS)
    w1h = w1h.astype(np.float32).astype(f8)
    w1h = np.ascontiguousarray(
        w1h.reshape(EPC, 128, NCH, 2 * I)).reshape(EPC, 128, NCH * 2 * I)
    w2h = (w2[lo:hi] * np.float32(WS)).astype(f8)
    w2h = np.ascontiguousarray(
        w2h.reshape(EPC, NCH, 128, D).transpose(0, 2, 1, 3)
    ).reshape(EPC, 128, NCH * D)
    gwp = (gate_w[perm].astype(np.float64) * nw[None, :]).T.astype(f16)

    m128 = np.zeros((128, 576), f16)
    m128[:, 0:320] = x2.astype(f16).reshape(128, NCH * T)
    m128[:, 320:480] = np.ascontiguousarray(gwp).reshape(128, NCH * E)
    m128[0:T, 480:544] = np.eye(T, dtype=f16)
    m128[0:T, 544:576] = np.tile(gate_b[perm][None, :], (T, 1)).astype(f16)

    b1s = (b1[lo:hi].astype(np.float32) * WS).astype(f16)
    m8t = np.zeros((8, 2688), f16)
    m8t[0:K, 0:512] = np.kron(np.eye(K), np.ones(128)).astype(f16)
    m8t[:, 512:1024] = np.kron(np.eye(8), np.ones(64)).astype(f16)
    m8t[:, 1024:1536] = b1s[:, :1024].reshape(EPC, 8, 128).transpose(
        1, 0, 2).reshape(8, EPC * 128)
    m8t[0:2, 1536:2048] = b1s[:, 1024:].reshape(EPC, 2, 128).transpose(
        1, 0, 2).reshape(2, EPC * 128)
    m8t[0:K, 2048:2688] = b2[lo:hi].astype(f16)
    return {"m128": m128, "m8t": m8t, "w1": w1h, "w2": w2h}


def kernel(**inputs):
    global LAST_EXEC_NS
    nc = _get_nc()
    from concourse.bass_utils import run_bass_kernel_spmd

    x = np.ascontiguousarray(np.asarray(inputs["x"], dtype=np.float32))
    norm_w = np.asarray(inputs["norm_w"], np.float32)
    gate_w = np.asarray(inputs["gate_w"], np.float32)
    gate_b = np.asarray(inputs["gate_b"], np.float32)
    w1 = np.asarray(inputs["w1"], np.float32)
    b1 = np.asarray(inputs["b1"], np.float32)
    w2 = np.asarray(inputs["w2"], np.float32)
    b2 = np.asarray(inputs["b2"], np.float32)

    x2 = np.ascontiguousarray(x[0, :, 0, :])  # (D, T)
    in_maps = [_prep_core(c, x2, norm_w, gate_w, gate_b, w1, b1, w2, b2)
               for c in range(NCORES)]

    res = run_bass_kernel_spmd(nc, in_maps, core_ids=list(range(NCORES)),
                               trace=TRACE, tmpdir=PROF_DIR)
    LAST_EXEC_NS = res.exec_time_ns
    total = np.zeros((D, T), np.float32)
    for r in res.results:
        o = np.asarray(r["out"], np.float32).reshape(128, NCH, T)
        total += o.transpose(1, 0, 2).reshape(D, T)
    return (x + total[None, :, None, :]).astype(np.float32)
